# revision 26
# baseline (speedup 1.0000x reference)
"""Trainium2 Bass kernel for nn_Attention (B=4, S=2048, D=1024, DK=256).

Computation (reference, per batch b):
    qp = q @ Wq.T            [S, DK]
    kp = q @ Wk.T            [S, DK]
    scores = qp @ kp.T / sqrt(DK)
    attn = softmax(scores, axis=-1)
    out = attn @ q           (v = q)
    y = out @ Wv.T           [S, D]

Sharding: 8 cores = 4 batches x 2 query-halves. Each core handles one batch's
full key/value sequence and one 1024-row query half. The host "rolls" the
sequence per core so that the core's query half occupies rows 0..1023; since
softmax is invariant to key permutation this changes nothing numerically.

All matmul operands are bf16 (PSUM accumulation stays fp32). Output y is bf16
(host casts back to fp32); pipeline rel-err ~5e-3 vs fp32 reference.

Startup is DMA-critical: the proj phase needs wk/wq + the first qt columns
ASAP, at ~460GB/s combined over two HWDGE queues (SP + ACT). The host packs a
per-queue "proj stream" (strmA even d-blocks, strmB odd) whose linear order =
consumption order, moved as 5 x 512KB descriptors per queue — big enough that
the tile framework's ~8-deep DMA-semaphore rotation never starves the queues
(the failure mode of many small DMAs: each dma_start blocks its issuing
engine until a rotation semaphore frees). Stream layout per queue, 2048-col
descriptors (tile rows = 128, bf16):
  A1 [wk(d0)|qt(d0)c0|wk(d2)|qt(d2)c0]   A2 same for d4,d6
  A3 [qt c1 for d0,d2,d4,d6]             A4 [qt c2+c3 for d0,d2]   A5 [d4,d6]
where wk(d) = 512 cols [wkT_d e0|e1 0:256 | wqT_d e0|e1 256:512], qt(d)cN =
512 cols of qT rows d*128..+128. The ACT-queue issues are emitted BEFORE the
exp-table warmup so the queue arms immediately. qn/wv ride behind (1MB each).

Proj runs d-major per query chunk: per qt tile d all 4 chunk matmuls
(kp e0,e1, qp e0,e1) run together, so each arriving 512KB descriptor unblocks
~3.5us of PE work. 10 dep-free warmup matmuls bridge boot -> first data and
absorb the DVFS ramp (~427ns/matmul until the clock ramps; the ramp restarts
after multi-us PE idle, so the stream must stay gapless).

Per-core dataflow after proj (kpT [e,s_k] and qpT [e,s_q] in SBUF bf16):
    per s_q chunk of 512:
      scoresT[s_k, s_q] = kpT.T @ qpT   (16 k-tiles x 2 e-acc)
      expT = exp(scoresT / 16)          (ScalarE, PSUM->SBUF bf16, fused scale)
      denom: DVE leaf adds (bf16 pairs -> fp32) + serial chain, pipelined
             against the exp stream; then ones-matmul partition sum ->
             PE-transpose 128-blocks -> reciprocal -> recip[s_q part, 1]
      unnormT[d, s_q] = qn.T @ expT     (8 d-tiles x 16 k-acc, 2 groups of 4)
      y[s_q, e_out] = unnormT.T @ wvT   (8 d-acc)
      y *= recip (per-partition, DVE, bf16 out) -> DMA out
Chunk-1 scores are dosed 4-at-a-time between unnorm/y batches to stay within
ACT's exp rate (823ns/tile vs 426ns to produce one).

PSUM (8 banks): tag "sc" bufs=4 (kp accs, score tiles, denom partition-sums),
tag "acc" bufs=4 (qp accs, unnorm groups, y tiles). A 4-deep score dose never
waits on the ACT exp drain with 4 rotating sc banks.
"""

import numpy as np
import ml_dtypes

import concourse.mybir as mybir
import concourse.tile as tile
from concourse import bacc
from concourse.bass_utils import run_bass_kernel_spmd
from concourse.masks import make_identity

B, S, D, DK = 4, 2048, 1024, 256
SQ = S // 2  # query rows per core
P = 128
CH = 512  # s_q chunk width
N_CORES = 8
WARMUP = 9

BF = mybir.dt.bfloat16
FR = mybir.dt.float32r
F32 = mybir.dt.float32
NPBF = ml_dtypes.bfloat16

KT = S // P  # 16 key tiles
DT = D // P  # 8 d tiles
ET = DK // P  # 2 e tiles

SW = 10240  # proj stream width (5 descriptors x 2048 cols)

_PROGRAM = None


def _stream_offsets(d):
    """(base column of wk chunk, qt c0, qt c1, qt c2/c3 base) for d-block d
    inside its queue's stream tile (blk = d//2 position within the stream)."""
    blk = d // 2
    wk = (blk // 2) * 2048 + (blk % 2) * 1024
    return wk, wk + 512, 4096 + blk * 512, 6144 + blk * 1024


def _build_program():
    nc = bacc.Bacc(None, target_bir_lowering=False, debug=False)

    sa_d = nc.dram_tensor("sa", [P, SW], BF, kind="ExternalInput")
    sb_d = nc.dram_tensor("sb", [P, SW], BF, kind="ExternalInput")
    qn_d = nc.dram_tensor("qn", [4 * P, 4096], BF, kind="ExternalInput")
    wvt_d = nc.dram_tensor("wvt", [2 * P, 4096], BF, kind="ExternalInput")
    y_d = nc.dram_tensor("y", [SQ, D], BF, kind="ExternalOutput")

    with tile.TileContext(nc) as tc:
        with (
            tc.tile_pool(name="pp", bufs=1) as pp,
            tc.tile_pool(name="ps", bufs=1, space="PSUM") as ps,
        ):
            # ---- warmup source: a single-partition row so the memset the
            # PE waits on is ~100ns (K=1 matmuls stream the same 512 cols,
            # so PE occupancy per warmup is unchanged) ----
            warm_r = pp.tile([1, 512], BF, tag="warm_r")
            nc.vector.memset(warm_r[:], 1.0)

            # ---- input tiles ----
            sa = pp.tile([P, SW], BF, tag="sa", name="sa")
            sb = pp.tile([P, SW], BF, tag="sb", name="sb")
            qn4 = [
                pp.tile([P, 4096], BF, tag="qn", bufs=4, name=f"qn{t}")
                for t in range(4)
            ]
            wv4 = [
                pp.tile([P, 4096], BF, tag="wvt", bufs=2, name=f"wv{j}")
                for j in range(2)
            ]

            # Input DMAs: 9 descriptors per queue, strict need order. The
            # first 512KB is split in two so the d0 weights+columns land
            # ~1.3us sooner (the early window is HBM-saturated ~130GB/s/q).
            bounds = [0, 1024, 2048, 4096, 6144, 8192, 10240]
            for lo, hi in zip(bounds[:-1], bounds[1:]):
                nc.sync.dma_start(sa[:, lo:hi], sa_d[:, lo:hi])
            for lo, hi in zip(bounds[:-1], bounds[1:]):
                nc.scalar.dma_start(sb[:, lo:hi], sb_d[:, lo:hi])
            for t in (0, 2):
                nc.sync.dma_start(qn4[t][:], qn_d[t * P : (t + 1) * P, :])
            for t in (1, 3):
                nc.scalar.dma_start(qn4[t][:], qn_d[t * P : (t + 1) * P, :])
            nc.sync.dma_start(wv4[0][:], wvt_d[0:P, :])
            nc.scalar.dma_start(wv4[1][:], wvt_d[P : 2 * P, :])

            # ---- warmup matmuls: bridge boot->first-data + DVFS ramp ----
            # 512-col matmuls (~427ns while throttled); sized to end at
            # first-data (~10.3us). A gap here resets the DVFS ramp (worse
            # than a slight overshoot), so err on the long side.
            pwarm = ps.tile([P, 512], F32, tag="sc", bufs=4, name="pwarm")
            for _ in range(WARMUP):
                nc.tensor.matmul(
                    pwarm[:], warm_r[:1, :P], warm_r[:], start=True, stop=True
                )

            # ---- constants (after warmup deps; all ready long before use) ----
            ones_f = pp.tile([P, 1], F32, tag="ones_f")
            nc.vector.memset(ones_f[:], 1.0)
            ones = pp.tile([P, 1], FR, tag="ones")
            nc.vector.tensor_copy(ones[:], ones_f[:])
            ident = pp.tile([P, P], F32, tag="ident")
            make_identity(nc, ident[:])
            # Warm the ACT exp table-set (~2.7us first-call cost). Emitted
            # after the scalar-queue DMA issues so the queue arms first.
            warm_act = pp.tile([P, 1], F32, tag="warm_act")
            nc.scalar.activation(
                warm_act[:], ones_f[:], mybir.ActivationFunctionType.Exp
            )

            # slicing helpers into the packed stream tiles
            def strm(d):
                return sa if d % 2 == 0 else sb

            def wk_sl(d, e):
                o = _stream_offsets(d)[0] + e * P
                return strm(d)[:, o : o + P]

            def wq_sl(d, e):
                o = _stream_offsets(d)[0] + 256 + e * P
                return strm(d)[:, o : o + P]

            def qt_sl(d, c):
                offs = _stream_offsets(d)
                if c == 0:
                    base = offs[1]
                elif c == 1:
                    base = offs[2]
                else:
                    base = offs[3] + (c - 2) * 512
                return strm(d)[:, base : base + 512]

            def qn_sl(k, d):
                base = (k % 4) * 1024 + d * P
                return qn4[k // 4][:, base : base + P]

            def wv_sl(d, n):
                base = (d % 4) * 1024 + n * 512
                return wv4[d // 4][:, base : base + 512]

            # ---- persistent on-chip intermediates ----
            kpt = [
                pp.tile([P, S], BF, tag="kpt", bufs=ET, name=f"kpt{e}")
                for e in range(ET)
            ]
            qpt = {
                (e, c): pp.tile([P, CH], BF, tag="qpt", bufs=ET * 2, name=f"qpt{e}_{c}")
                for e in range(ET)
                for c in range(2)
            }
            expt = {}  # (chunk, k) -> bf16 tile, allocated on the fly

            # ---- helpers ----
            def proj_chunk(n, with_qp):
                """kp (and qp) for query/key chunk n, d-major: per qt tile d
                all chunk matmuls run together."""
                pks = [
                    ps.tile([P, 512], F32, tag="sc", bufs=4, name=f"pk{e}_{n}")
                    for e in range(ET)
                ]
                pqs = (
                    [
                        ps.tile([P, 512], F32, tag="acc", bufs=4, name=f"pq{e}_{n}")
                        for e in range(ET)
                    ]
                    if with_qp
                    else []
                )
                for d in range(DT):
                    rhs = qt_sl(d, n)
                    for e in range(ET):
                        nc.tensor.matmul(
                            pks[e][:], wk_sl(d, e), rhs,
                            start=(d == 0), stop=(d == DT - 1),
                        )
                    for e in range(ET):
                        if with_qp:
                            nc.tensor.matmul(
                                pqs[e][:], wq_sl(d, e), rhs,
                                start=(d == 0), stop=(d == DT - 1),
                            )
                for e in range(ET):
                    nc.vector.tensor_copy(kpt[e][:, n * 512 : (n + 1) * 512], pks[e][:])
                for e in range(ET):
                    if with_qp:
                        nc.vector.tensor_copy(qpt[e, n][:], pqs[e][:])

            def scores_block(c, ks):
                """scoresT + exp for key tiles ks of chunk c."""
                for k in ks:
                    sc = ps.tile([P, CH], F32, tag="sc", bufs=4, name=f"sc{c}_{k}")
                    for e in range(ET):
                        nc.tensor.matmul(
                            sc[:],
                            kpt[e][:, k * P : (k + 1) * P],
                            qpt[e, c][:],
                            start=(e == 0),
                            stop=(e == ET - 1),
                        )
                    ex = pp.tile([P, CH], BF, tag="expt", bufs=20, name=f"ex{c}_{k}")
                    nc.scalar.activation(
                        ex[:], sc[:], mybir.ActivationFunctionType.Exp, scale=1.0 / 16.0
                    )
                    expt[c, k] = ex

            def denom_dve(c):
                """DVE part of the denominator: leaf adds pipelined with the
                exp stream, serial fp32 chain tracking them."""
                lvl = [
                    pp.tile([P, CH], F32, tag="dtree", bufs=8, name=f"dt{c}_{i}")
                    for i in range(8)
                ]
                for i in range(8):
                    nc.vector.tensor_tensor(
                        lvl[i][:],
                        expt[c, 2 * i][:],
                        expt[c, 2 * i + 1][:],
                        op=mybir.AluOpType.add,
                    )
                    if i > 0:
                        nc.vector.tensor_tensor(
                            lvl[0][:], lvl[0][:], lvl[i][:], op=mybir.AluOpType.add
                        )
                daccr = pp.tile([P, CH], FR, tag="daccr", bufs=2, name=f"daccr{c}")
                nc.vector.tensor_copy(daccr[:], lvl[0][:])
                return daccr

            def denom_pe(c, daccr):
                """PE part (partition-sum + transpose): emitted later in the
                PE stream so it never head-of-line blocks on the DVE tree."""
                pd = ps.tile([1, CH], F32, tag="sc", bufs=4, name=f"pd{c}")
                nc.tensor.matmul(pd[:], ones[:], daccr[:], start=True, stop=True)
                drow = pp.tile([1, CH], F32, tag="drow", bufs=2, name=f"drow{c}")
                # drow/recip on ACT: pd/pt occupy score-dose "sc" slots, and
                # later score matmuls wait on these readers — ACT retires
                # them promptly (DVE is busy with unnorm evacuations here).
                nc.scalar.activation(
                    drow[:], pd[:], mybir.ActivationFunctionType.Copy
                )
                pt = ps.tile([P, CH // P], F32, tag="sc", bufs=4, name=f"pt{c}")
                for j in range(CH // P):
                    nc.tensor.transpose(
                        pt[:, j : j + 1], drow[:1, j * P : (j + 1) * P], ident[:1, :1]
                    )
                recip = pp.tile([P, CH // P], F32, tag="recip", bufs=2, name=f"recip{c}")
                nc.vector.reciprocal(recip[:], pt[:])
                return recip

            def unnorm_group(c, g, unsb):
                accs = [
                    ps.tile([P, CH], F32, tag="acc", bufs=4, name=f"un{c}_{g}_{i}")
                    for i in range(4)
                ]
                for k in range(KT):
                    for i in range(4):
                        d = g * 4 + i
                        nc.tensor.matmul(
                            accs[i][:],
                            qn_sl(k, d),
                            expt[c, k][:],
                            start=(k == 0),
                            stop=(k == KT - 1),
                        )
                for i in range(4):
                    us = pp.tile([P, CH], BF, tag="unsb", bufs=8, name=f"us{c}_{g}_{i}")
                    # Odd copies ride ACT (has PSUM access, slack between
                    # exp doses) so the y phase never waits on DVE.
                    if i % 2:
                        nc.scalar.activation(
                            us[:], accs[i][:], mybir.ActivationFunctionType.Copy
                        )
                    else:
                        nc.vector.tensor_copy(us[:], accs[i][:])
                    unsb.append(us)

            def y_ms(c, unsb, recip, ms, split_last=False):
                cs = c * CH
                for m in ms:
                    for n in range(D // 512):
                        yb = ps.tile([P, 512], F32, tag="acc", bufs=4, name=f"yb{c}_{m}_{n}")
                        for d in range(DT):
                            nc.tensor.matmul(
                                yb[:],
                                unsb[d][:, m * P : (m + 1) * P],
                                wv_sl(d, n),
                                start=(d == 0),
                                stop=(d == DT - 1),
                            )
                        last = split_last and m == ms[-1] and n == D // 512 - 1
                        # Final block: 256+256 split with one DMA issue per
                        # engine queue so the two 600ns issues overlap.
                        pieces = ((0, 256, nc.sync), (256, 256, nc.scalar)) if last \
                            else ((0, 512, nc.sync),)
                        for h, (off, hw, eng) in enumerate(pieces):
                            ys = pp.tile(
                                [P, hw], BF, tag="ysb", bufs=8, name=f"ys{c}_{m}_{n}_{h}"
                            )
                            # Final block's second mul on ACT (Copy with
                            # per-partition scale): the two halves scale in
                            # parallel instead of serializing on DVE at the
                            # kernel tail.
                            if h:
                                nc.scalar.activation(
                                    ys[:],
                                    yb[:, off : off + hw],
                                    mybir.ActivationFunctionType.Copy,
                                    scale=recip[:, m : m + 1],
                                )
                            else:
                                nc.vector.tensor_scalar_mul(
                                    ys[:], yb[:, off : off + hw], recip[:, m : m + 1]
                                )
                            eng.dma_start(
                                y_d[
                                    cs + m * P : cs + (m + 1) * P,
                                    n * 512 + off : n * 512 + off + hw,
                                ],
                                ys[:],
                            )

            # ---- schedule (trace order == PE priority order) ----
            proj_chunk(0, with_qp=True)
            proj_chunk(1, with_qp=True)
            scores_block(0, range(0, 4))
            proj_chunk(2, with_qp=False)
            scores_block(0, range(4, 8))
            proj_chunk(3, with_qp=False)
            scores_block(0, range(8, 12))
            scores_block(0, range(12, 16))
            daccr0 = denom_dve(0)
            unsb0 = []
            unnorm_group(0, 0, unsb0)
            scores_block(1, range(0, 4))
            recip0 = denom_pe(0, daccr0)
            unnorm_group(0, 1, unsb0)
            scores_block(1, range(4, 8))
            y_ms(0, unsb0, recip0, (0, 1))
            scores_block(1, range(8, 12))
            y_ms(0, unsb0, recip0, (2, 3))
            scores_block(1, range(12, 16))
            daccr1 = denom_dve(1)
            unsb1 = []
            unnorm_group(1, 0, unsb1)
            recip1 = denom_pe(1, daccr1)
            unnorm_group(1, 1, unsb1)
            y_ms(1, unsb1, recip1, (0, 1, 2, 3), split_last=True)

    nc.compile()
    return nc


def build_in_maps(q, Wq, Wk, Wv):
    q = np.asarray(q, dtype=np.float32)

    wqt = np.asarray(Wq, dtype=np.float32).T.astype(NPBF)  # [D, DK]
    wkt = np.asarray(Wk, dtype=np.float32).T.astype(NPBF)
    wkq_full = np.concatenate([wkt, wqt], axis=1)  # [D, 512] rows d*128+p
    # wvt tile J: [128, 4096] cols (d%4)*1024 + e, d=4J..4J+3
    wvT = np.asarray(Wv, dtype=np.float32).T.astype(NPBF)  # [D, D]
    wvt = np.ascontiguousarray(
        wvT.reshape(2, 4, P, 1024).transpose(0, 2, 1, 3).reshape(2 * P, 4096)
    )

    def pack_stream(qtT, ds):
        """[128, SW] stream for d-blocks ds: A1[wk qtc0 x2] A2[same] A3[qt c1
        x4] A4[qt c2c3 x2] A5[same] — linear order == consumption order."""
        blocks = []
        for pair in (ds[0:2], ds[2:4]):  # A1, A2
            for d in pair:
                blocks.append(wkq_full[d * P : (d + 1) * P, :])  # [128,512]
                blocks.append(qtT[d * P : (d + 1) * P, 0:512])
        for d in ds:  # A3
            blocks.append(qtT[d * P : (d + 1) * P, 512:1024])
        for d in ds:  # A4, A5
            blocks.append(qtT[d * P : (d + 1) * P, 1024:2048])
        return np.ascontiguousarray(np.concatenate(blocks, axis=1))

    in_maps = []
    for core in range(N_CORES):
        b, h = divmod(core, 2)
        qb = q[b]
        rolled = np.concatenate(
            [qb[h * SQ : (h + 1) * SQ], qb[(1 - h) * SQ : (2 - h) * SQ]]
        ).astype(NPBF)
        qtT = np.ascontiguousarray(rolled.T)  # [D, S]
        # qn tile t: [128, 4096] cols (k%4)*1024 + d, k=4t..4t+3
        qn_packed = np.ascontiguousarray(
            rolled.reshape(4, 4, P, D).transpose(0, 2, 1, 3).reshape(4 * P, 4096)
        )
        in_maps.append(
            {
                "sa": pack_stream(qtT, (0, 2, 4, 6)),
                "sb": pack_stream(qtT, (1, 3, 5, 7)),
                "qn": qn_packed,
                "wvt": wvt,
            }
        )
    return in_maps


def kernel(q, Wq, Wk, Wv):
    global _PROGRAM
    if _PROGRAM is None:
        _PROGRAM = _build_program()
    nc = _PROGRAM
    in_maps = build_in_maps(q, Wq, Wk, Wv)
    res = run_bass_kernel_spmd(nc, in_maps, list(range(N_CORES)))

    out = np.empty((B, S, D), dtype=np.float32)
    for core in range(N_CORES):
        b, h = divmod(core, 2)
        out[b, h * SQ : (h + 1) * SQ, :] = np.asarray(
            res.results[core]["y"], dtype=np.float32
        )
    return out


# revision 29
# speedup vs baseline: 1.0047x; 1.0047x over previous
"""Trainium2 Bass kernel for nn_Attention (B=4, S=2048, D=1024, DK=256).

Computation (reference, per batch b):
    qp = q @ Wq.T            [S, DK]
    kp = q @ Wk.T            [S, DK]
    scores = qp @ kp.T / sqrt(DK)
    attn = softmax(scores, axis=-1)
    out = attn @ q           (v = q)
    y = out @ Wv.T           [S, D]

Sharding: 8 cores = 4 batches x 2 query-halves. Each core handles one batch's
full key/value sequence and one 1024-row query half. The host "rolls" the
sequence per core so that the core's query half occupies rows 0..1023; since
softmax is invariant to key permutation this changes nothing numerically.

All matmul operands are bf16 (PSUM accumulation stays fp32). Output y is bf16
(host casts back to fp32); pipeline rel-err ~5e-3 vs fp32 reference.

Startup is DMA-critical: the proj phase needs wk/wq + the first qt columns
ASAP, at ~460GB/s combined over two HWDGE queues (SP + ACT). The host packs a
per-queue "proj stream" (strmA even d-blocks, strmB odd) whose linear order =
consumption order, moved as 5 x 512KB descriptors per queue — big enough that
the tile framework's ~8-deep DMA-semaphore rotation never starves the queues
(the failure mode of many small DMAs: each dma_start blocks its issuing
engine until a rotation semaphore frees). Stream layout per queue, 2048-col
descriptors (tile rows = 128, bf16):
  A1 [wk(d0)|qt(d0)c0|wk(d2)|qt(d2)c0]   A2 same for d4,d6
  A3 [qt c1 for d0,d2,d4,d6]             A4 [qt c2+c3 for d0,d2]   A5 [d4,d6]
where wk(d) = 512 cols [wkT_d e0|e1 0:256 | wqT_d e0|e1 256:512], qt(d)cN =
512 cols of qT rows d*128..+128. The ACT-queue issues are emitted BEFORE the
exp-table warmup so the queue arms immediately. qn/wv ride behind (1MB each).

Proj runs d-major per query chunk: per qt tile d all 4 chunk matmuls
(kp e0,e1, qp e0,e1) run together, so each arriving 512KB descriptor unblocks
~3.5us of PE work. 10 dep-free warmup matmuls bridge boot -> first data and
absorb the DVFS ramp (~427ns/matmul until the clock ramps; the ramp restarts
after multi-us PE idle, so the stream must stay gapless).

Per-core dataflow after proj (kpT [e,s_k] and qpT [e,s_q] in SBUF bf16):
    per s_q chunk of 512:
      scoresT[s_k, s_q] = kpT.T @ qpT   (16 k-tiles x 2 e-acc)
      expT = exp(scoresT / 16)          (ScalarE, PSUM->SBUF bf16, fused scale)
      denom: DVE leaf adds (bf16 pairs -> fp32) + serial chain, pipelined
             against the exp stream; then ones-matmul partition sum ->
             PE-transpose 128-blocks -> reciprocal -> recip[s_q part, 1]
      unnormT[d, s_q] = qn.T @ expT     (8 d-tiles x 16 k-acc, 2 groups of 4)
      y[s_q, e_out] = unnormT.T @ wvT   (8 d-acc)
      y *= recip (per-partition, DVE, bf16 out) -> DMA out
Chunk-1 scores are dosed 4-at-a-time between unnorm/y batches to stay within
ACT's exp rate (823ns/tile vs 426ns to produce one).

PSUM (8 banks): tag "sc" bufs=4 (kp accs, score tiles, denom partition-sums),
tag "acc" bufs=4 (qp accs, unnorm groups, y tiles). A 4-deep score dose never
waits on the ACT exp drain with 4 rotating sc banks.
"""

import numpy as np
import ml_dtypes

import concourse.mybir as mybir
import concourse.tile as tile
from concourse import bacc
from concourse.bass_utils import run_bass_kernel_spmd
from concourse.masks import make_identity

B, S, D, DK = 4, 2048, 1024, 256
SQ = S // 2  # query rows per core
P = 128
CH = 512  # s_q chunk width
N_CORES = 8
WARMUP = 8

BF = mybir.dt.bfloat16
FR = mybir.dt.float32r
F32 = mybir.dt.float32
NPBF = ml_dtypes.bfloat16

KT = S // P  # 16 key tiles
DT = D // P  # 8 d tiles
ET = DK // P  # 2 e tiles

SW = 10240  # proj stream width (5 descriptors x 2048 cols)

_PROGRAM = None


def _stream_offsets(d):
    """(base column of wk chunk, qt c0, qt c1, qt c2/c3 base) for d-block d
    inside its queue's stream tile (blk = d//2 position within the stream)."""
    blk = d // 2
    wk = (blk // 2) * 2048 + (blk % 2) * 1024
    return wk, wk + 512, 4096 + blk * 512, 6144 + blk * 1024


def _build_program():
    nc = bacc.Bacc(None, target_bir_lowering=False, debug=False)

    sa_d = nc.dram_tensor("sa", [P, SW], BF, kind="ExternalInput")
    sb_d = nc.dram_tensor("sb", [P, SW], BF, kind="ExternalInput")
    qn_d = nc.dram_tensor("qn", [4 * P, 4096], BF, kind="ExternalInput")
    wvt_d = nc.dram_tensor("wvt", [2 * P, 4096], BF, kind="ExternalInput")
    y_d = nc.dram_tensor("y", [SQ, D], BF, kind="ExternalOutput")

    with tile.TileContext(nc) as tc:
        with (
            tc.tile_pool(name="pp", bufs=1) as pp,
            tc.tile_pool(name="ps", bufs=1, space="PSUM") as ps,
        ):
            # ---- warmup source: full 128 partitions — K=1 warmups do NOT
            # exercise the PE enough to trigger the DVFS ramp (measured:
            # real matmuls then run at half clock for ~10 instructions) ----
            warm_r = pp.tile([P, 512], BF, tag="warm_r")
            nc.vector.memset(warm_r[:], 1.0)

            # ---- input tiles ----
            sa = pp.tile([P, SW], BF, tag="sa", name="sa")
            sb = pp.tile([P, SW], BF, tag="sb", name="sb")
            qn4 = [
                pp.tile([P, 4096], BF, tag="qn", bufs=4, name=f"qn{t}")
                for t in range(4)
            ]
            wv4 = [
                pp.tile([P, 4096], BF, tag="wvt", bufs=2, name=f"wv{j}")
                for j in range(2)
            ]

            # Input DMAs: 9 descriptors per queue, strict need order. The
            # first 512KB is split in two so the d0 weights+columns land
            # ~1.3us sooner (the early window is HBM-saturated ~130GB/s/q).
            bounds = [0, 1024, 2048, 4096, 6144, 8192, 10240]
            for lo, hi in zip(bounds[:-1], bounds[1:]):
                nc.sync.dma_start(sa[:, lo:hi], sa_d[:, lo:hi])
            for lo, hi in zip(bounds[:-1], bounds[1:]):
                nc.scalar.dma_start(sb[:, lo:hi], sb_d[:, lo:hi])
            for t in (0, 2):
                nc.sync.dma_start(qn4[t][:], qn_d[t * P : (t + 1) * P, :])
            for t in (1, 3):
                nc.scalar.dma_start(qn4[t][:], qn_d[t * P : (t + 1) * P, :])
            nc.sync.dma_start(wv4[0][:], wvt_d[0:P, :])
            nc.scalar.dma_start(wv4[1][:], wvt_d[P : 2 * P, :])

            # ---- warmup matmuls: bridge boot->first-data + DVFS ramp ----
            # 512-col matmuls (~427ns while throttled); sized to end at
            # first-data (~10.3us). A gap here resets the DVFS ramp (worse
            # than a slight overshoot), so err on the long side.
            pwarm = ps.tile([P, 512], F32, tag="sc", bufs=4, name="pwarm")
            for _ in range(WARMUP):
                nc.tensor.matmul(
                    pwarm[:], warm_r[:, :P], warm_r[:], start=True, stop=True
                )

            # ---- constants (after warmup deps; all ready long before use) ----
            ones_f = pp.tile([P, 1], F32, tag="ones_f")
            nc.vector.memset(ones_f[:], 1.0)
            ones = pp.tile([P, 1], FR, tag="ones")
            nc.vector.tensor_copy(ones[:], ones_f[:])
            ident = pp.tile([P, P], F32, tag="ident")
            make_identity(nc, ident[:])
            # Warm the ACT exp table-set (~2.7us first-call cost). Emitted
            # after the scalar-queue DMA issues so the queue arms first.
            warm_act = pp.tile([P, 1], F32, tag="warm_act")
            nc.scalar.activation(
                warm_act[:], ones_f[:], mybir.ActivationFunctionType.Exp
            )

            # slicing helpers into the packed stream tiles
            def strm(d):
                return sa if d % 2 == 0 else sb

            def wk_sl(d, e):
                o = _stream_offsets(d)[0] + e * P
                return strm(d)[:, o : o + P]

            def wq_sl(d, e):
                o = _stream_offsets(d)[0] + 256 + e * P
                return strm(d)[:, o : o + P]

            def qt_sl(d, c):
                offs = _stream_offsets(d)
                if c == 0:
                    base = offs[1]
                elif c == 1:
                    base = offs[2]
                else:
                    base = offs[3] + (c - 2) * 512
                return strm(d)[:, base : base + 512]

            def qn_sl(k, d):
                base = (k % 4) * 1024 + d * P
                return qn4[k // 4][:, base : base + P]

            def wv_sl(d, n):
                base = (d % 4) * 1024 + n * 512
                return wv4[d // 4][:, base : base + 512]

            # ---- persistent on-chip intermediates ----
            kpt = [
                pp.tile([P, S], BF, tag="kpt", bufs=ET, name=f"kpt{e}")
                for e in range(ET)
            ]
            qpt = {
                (e, c): pp.tile([P, CH], BF, tag="qpt", bufs=ET * 2, name=f"qpt{e}_{c}")
                for e in range(ET)
                for c in range(2)
            }
            expt = {}  # (chunk, k) -> bf16 tile, allocated on the fly

            # ---- helpers ----
            def proj_chunk(n, with_qp):
                """kp (and qp) for query/key chunk n, d-major: per qt tile d
                all chunk matmuls run together."""
                pks = [
                    ps.tile([P, 512], F32, tag="sc", bufs=4, name=f"pk{e}_{n}")
                    for e in range(ET)
                ]
                pqs = (
                    [
                        ps.tile([P, 512], F32, tag="acc", bufs=4, name=f"pq{e}_{n}")
                        for e in range(ET)
                    ]
                    if with_qp
                    else []
                )
                for d in range(DT):
                    rhs = qt_sl(d, n)
                    for e in range(ET):
                        nc.tensor.matmul(
                            pks[e][:], wk_sl(d, e), rhs,
                            start=(d == 0), stop=(d == DT - 1),
                        )
                    for e in range(ET):
                        if with_qp:
                            nc.tensor.matmul(
                                pqs[e][:], wq_sl(d, e), rhs,
                                start=(d == 0), stop=(d == DT - 1),
                            )
                for e in range(ET):
                    nc.vector.tensor_copy(kpt[e][:, n * 512 : (n + 1) * 512], pks[e][:])
                for e in range(ET):
                    if with_qp:
                        nc.vector.tensor_copy(qpt[e, n][:], pqs[e][:])

            def scores_block(c, ks):
                """scoresT + exp for key tiles ks of chunk c."""
                for k in ks:
                    sc = ps.tile([P, CH], F32, tag="sc", bufs=4, name=f"sc{c}_{k}")
                    for e in range(ET):
                        nc.tensor.matmul(
                            sc[:],
                            kpt[e][:, k * P : (k + 1) * P],
                            qpt[e, c][:],
                            start=(e == 0),
                            stop=(e == ET - 1),
                        )
                    ex = pp.tile([P, CH], BF, tag="expt", bufs=20, name=f"ex{c}_{k}")
                    nc.scalar.activation(
                        ex[:], sc[:], mybir.ActivationFunctionType.Exp, scale=1.0 / 16.0
                    )
                    expt[c, k] = ex

            def denom_dve(c):
                """DVE part of the denominator: leaf adds pipelined with the
                exp stream, serial fp32 chain tracking them."""
                lvl = [
                    pp.tile([P, CH], F32, tag="dtree", bufs=8, name=f"dt{c}_{i}")
                    for i in range(8)
                ]
                for i in range(8):
                    nc.vector.tensor_tensor(
                        lvl[i][:],
                        expt[c, 2 * i][:],
                        expt[c, 2 * i + 1][:],
                        op=mybir.AluOpType.add,
                    )
                    if i > 0:
                        nc.vector.tensor_tensor(
                            lvl[0][:], lvl[0][:], lvl[i][:], op=mybir.AluOpType.add
                        )
                daccr = pp.tile([P, CH], FR, tag="daccr", bufs=2, name=f"daccr{c}")
                nc.vector.tensor_copy(daccr[:], lvl[0][:])
                return daccr

            def denom_pe(c, daccr):
                """PE part (partition-sum + transpose): emitted later in the
                PE stream so it never head-of-line blocks on the DVE tree."""
                pd = ps.tile([1, CH], F32, tag="sc", bufs=4, name=f"pd{c}")
                nc.tensor.matmul(pd[:], ones[:], daccr[:], start=True, stop=True)
                drow = pp.tile([1, CH], F32, tag="drow", bufs=2, name=f"drow{c}")
                # drow/recip on ACT: pd/pt occupy score-dose "sc" slots, and
                # later score matmuls wait on these readers — ACT retires
                # them promptly (DVE is busy with unnorm evacuations here).
                nc.scalar.activation(
                    drow[:], pd[:], mybir.ActivationFunctionType.Copy
                )
                pt = ps.tile([P, CH // P], F32, tag="sc", bufs=4, name=f"pt{c}")
                for j in range(CH // P):
                    nc.tensor.transpose(
                        pt[:, j : j + 1], drow[:1, j * P : (j + 1) * P], ident[:1, :1]
                    )
                recip = pp.tile([P, CH // P], F32, tag="recip", bufs=2, name=f"recip{c}")
                nc.vector.reciprocal(recip[:], pt[:])
                return recip

            def unnorm_group(c, g, unsb):
                accs = [
                    ps.tile([P, CH], F32, tag="acc", bufs=4, name=f"un{c}_{g}_{i}")
                    for i in range(4)
                ]
                for k in range(KT):
                    for i in range(4):
                        d = g * 4 + i
                        nc.tensor.matmul(
                            accs[i][:],
                            qn_sl(k, d),
                            expt[c, k][:],
                            start=(k == 0),
                            stop=(k == KT - 1),
                        )
                for i in range(4):
                    us = pp.tile([P, CH], BF, tag="unsb", bufs=8, name=f"us{c}_{g}_{i}")
                    # Odd copies ride ACT (has PSUM access, slack between
                    # exp doses) so the y phase never waits on DVE.
                    if i % 2:
                        nc.scalar.activation(
                            us[:], accs[i][:], mybir.ActivationFunctionType.Copy
                        )
                    else:
                        nc.vector.tensor_copy(us[:], accs[i][:])
                    unsb.append(us)

            def y_ms(c, unsb, recip, ms, split_last=False):
                cs = c * CH
                for m in ms:
                    for n in range(D // 512):
                        yb = ps.tile([P, 512], F32, tag="acc", bufs=4, name=f"yb{c}_{m}_{n}")
                        for d in range(DT):
                            nc.tensor.matmul(
                                yb[:],
                                unsb[d][:, m * P : (m + 1) * P],
                                wv_sl(d, n),
                                start=(d == 0),
                                stop=(d == DT - 1),
                            )
                        last = split_last and m == ms[-1] and n == D // 512 - 1
                        # Final block: 256+256 split with one DMA issue per
                        # engine queue so the two 600ns issues overlap.
                        pieces = ((0, 256, nc.sync), (256, 256, nc.scalar)) if last \
                            else ((0, 512, nc.sync),)
                        for h, (off, hw, eng) in enumerate(pieces):
                            ys = pp.tile(
                                [P, hw], BF, tag="ysb", bufs=8, name=f"ys{c}_{m}_{n}_{h}"
                            )
                            # Final block's second mul on ACT (Copy with
                            # per-partition scale): the two halves scale in
                            # parallel instead of serializing on DVE at the
                            # kernel tail.
                            if h:
                                nc.scalar.activation(
                                    ys[:],
                                    yb[:, off : off + hw],
                                    mybir.ActivationFunctionType.Copy,
                                    scale=recip[:, m : m + 1],
                                )
                            else:
                                nc.vector.tensor_scalar_mul(
                                    ys[:], yb[:, off : off + hw], recip[:, m : m + 1]
                                )
                            eng.dma_start(
                                y_d[
                                    cs + m * P : cs + (m + 1) * P,
                                    n * 512 + off : n * 512 + off + hw,
                                ],
                                ys[:],
                            )

            # ---- schedule (trace order == PE priority order) ----
            proj_chunk(0, with_qp=True)
            proj_chunk(1, with_qp=True)
            scores_block(0, range(0, 4))
            proj_chunk(2, with_qp=False)
            scores_block(0, range(4, 8))
            proj_chunk(3, with_qp=False)
            scores_block(0, range(8, 12))
            scores_block(0, range(12, 16))
            daccr0 = denom_dve(0)
            unsb0 = []
            unnorm_group(0, 0, unsb0)
            scores_block(1, range(0, 4))
            recip0 = denom_pe(0, daccr0)
            unnorm_group(0, 1, unsb0)
            scores_block(1, range(4, 8))
            y_ms(0, unsb0, recip0, (0, 1))
            scores_block(1, range(8, 12))
            y_ms(0, unsb0, recip0, (2, 3))
            scores_block(1, range(12, 16))
            daccr1 = denom_dve(1)
            unsb1 = []
            unnorm_group(1, 0, unsb1)
            recip1 = denom_pe(1, daccr1)
            unnorm_group(1, 1, unsb1)
            y_ms(1, unsb1, recip1, (0, 1, 2, 3), split_last=True)

    nc.compile()
    return nc


def build_in_maps(q, Wq, Wk, Wv):
    q = np.asarray(q, dtype=np.float32)

    wqt = np.asarray(Wq, dtype=np.float32).T.astype(NPBF)  # [D, DK]
    wkt = np.asarray(Wk, dtype=np.float32).T.astype(NPBF)
    wkq_full = np.concatenate([wkt, wqt], axis=1)  # [D, 512] rows d*128+p
    # wvt tile J: [128, 4096] cols (d%4)*1024 + e, d=4J..4J+3
    wvT = np.asarray(Wv, dtype=np.float32).T.astype(NPBF)  # [D, D]
    wvt = np.ascontiguousarray(
        wvT.reshape(2, 4, P, 1024).transpose(0, 2, 1, 3).reshape(2 * P, 4096)
    )

    def pack_stream(qtT, ds):
        """[128, SW] stream for d-blocks ds: A1[wk qtc0 x2] A2[same] A3[qt c1
        x4] A4[qt c2c3 x2] A5[same] — linear order == consumption order."""
        blocks = []
        for pair in (ds[0:2], ds[2:4]):  # A1, A2
            for d in pair:
                blocks.append(wkq_full[d * P : (d + 1) * P, :])  # [128,512]
                blocks.append(qtT[d * P : (d + 1) * P, 0:512])
        for d in ds:  # A3
            blocks.append(qtT[d * P : (d + 1) * P, 512:1024])
        for d in ds:  # A4, A5
            blocks.append(qtT[d * P : (d + 1) * P, 1024:2048])
        return np.ascontiguousarray(np.concatenate(blocks, axis=1))

    in_maps = []
    for core in range(N_CORES):
        b, h = divmod(core, 2)
        qb = q[b]
        rolled = np.concatenate(
            [qb[h * SQ : (h + 1) * SQ], qb[(1 - h) * SQ : (2 - h) * SQ]]
        ).astype(NPBF)
        qtT = np.ascontiguousarray(rolled.T)  # [D, S]
        # qn tile t: [128, 4096] cols (k%4)*1024 + d, k=4t..4t+3
        qn_packed = np.ascontiguousarray(
            rolled.reshape(4, 4, P, D).transpose(0, 2, 1, 3).reshape(4 * P, 4096)
        )
        in_maps.append(
            {
                "sa": pack_stream(qtT, (0, 2, 4, 6)),
                "sb": pack_stream(qtT, (1, 3, 5, 7)),
                "qn": qn_packed,
                "wvt": wvt,
            }
        )
    return in_maps


def kernel(q, Wq, Wk, Wv):
    global _PROGRAM
    if _PROGRAM is None:
        _PROGRAM = _build_program()
    nc = _PROGRAM
    in_maps = build_in_maps(q, Wq, Wk, Wv)
    res = run_bass_kernel_spmd(nc, in_maps, list(range(N_CORES)))

    out = np.empty((B, S, D), dtype=np.float32)
    for core in range(N_CORES):
        b, h = divmod(core, 2)
        out[b, h * SQ : (h + 1) * SQ, :] = np.asarray(
            res.results[core]["y"], dtype=np.float32
        )
    return out


# revision 30
# speedup vs baseline: 1.0103x; 1.0056x over previous
"""Trainium2 Bass kernel for nn_Attention (B=4, S=2048, D=1024, DK=256).

Computation (reference, per batch b):
    qp = q @ Wq.T            [S, DK]
    kp = q @ Wk.T            [S, DK]
    scores = qp @ kp.T / sqrt(DK)
    attn = softmax(scores, axis=-1)
    out = attn @ q           (v = q)
    y = out @ Wv.T           [S, D]

Sharding: 8 cores = 4 batches x 2 query-halves. Each core handles one batch's
full key/value sequence and one 1024-row query half. The host "rolls" the
sequence per core so that the core's query half occupies rows 0..1023; since
softmax is invariant to key permutation this changes nothing numerically.

All matmul operands are bf16 (PSUM accumulation stays fp32). Output y is bf16
(host casts back to fp32); pipeline rel-err ~5e-3 vs fp32 reference.

Startup is DMA-critical: the proj phase needs wk/wq + the first qt columns
ASAP, at ~460GB/s combined over two HWDGE queues (SP + ACT). The host packs a
per-queue "proj stream" (strmA even d-blocks, strmB odd) whose linear order =
consumption order, moved as 5 x 512KB descriptors per queue — big enough that
the tile framework's ~8-deep DMA-semaphore rotation never starves the queues
(the failure mode of many small DMAs: each dma_start blocks its issuing
engine until a rotation semaphore frees). Stream layout per queue, 2048-col
descriptors (tile rows = 128, bf16):
  A1 [wk(d0)|qt(d0)c0|wk(d2)|qt(d2)c0]   A2 same for d4,d6
  A3 [qt c1 for d0,d2,d4,d6]             A4 [qt c2+c3 for d0,d2]   A5 [d4,d6]
where wk(d) = 512 cols [wkT_d e0|e1 0:256 | wqT_d e0|e1 256:512], qt(d)cN =
512 cols of qT rows d*128..+128. The ACT-queue issues are emitted BEFORE the
exp-table warmup so the queue arms immediately. qn/wv ride behind (1MB each).

Proj runs d-major per query chunk: per qt tile d all 4 chunk matmuls
(kp e0,e1, qp e0,e1) run together, so each arriving 512KB descriptor unblocks
~3.5us of PE work. 10 dep-free warmup matmuls bridge boot -> first data and
absorb the DVFS ramp (~427ns/matmul until the clock ramps; the ramp restarts
after multi-us PE idle, so the stream must stay gapless).

Per-core dataflow after proj (kpT [e,s_k] and qpT [e,s_q] in SBUF bf16):
    per s_q chunk of 512:
      scoresT[s_k, s_q] = kpT.T @ qpT   (16 k-tiles x 2 e-acc)
      expT = exp(scoresT / 16)          (ScalarE, PSUM->SBUF bf16, fused scale)
      denom: DVE leaf adds (bf16 pairs -> fp32) + serial chain, pipelined
             against the exp stream; then ones-matmul partition sum ->
             PE-transpose 128-blocks -> reciprocal -> recip[s_q part, 1]
      unnormT[d, s_q] = qn.T @ expT     (8 d-tiles x 16 k-acc, 2 groups of 4)
      y[s_q, e_out] = unnormT.T @ wvT   (8 d-acc)
      y *= recip (per-partition, DVE, bf16 out) -> DMA out
Chunk-1 scores are dosed 4-at-a-time between unnorm/y batches to stay within
ACT's exp rate (823ns/tile vs 426ns to produce one).

PSUM (8 banks): tag "sc" bufs=4 (kp accs, score tiles, denom partition-sums),
tag "acc" bufs=4 (qp accs, unnorm groups, y tiles). A 4-deep score dose never
waits on the ACT exp drain with 4 rotating sc banks.
"""

import numpy as np
import ml_dtypes

import concourse.mybir as mybir
import concourse.tile as tile
from concourse import bacc
from concourse.bass_utils import run_bass_kernel_spmd
from concourse.masks import make_identity

B, S, D, DK = 4, 2048, 1024, 256
SQ = S // 2  # query rows per core
P = 128
CH = 512  # s_q chunk width
N_CORES = 8
WARMUP = 8

BF = mybir.dt.bfloat16
FR = mybir.dt.float32r
F32 = mybir.dt.float32
NPBF = ml_dtypes.bfloat16

KT = S // P  # 16 key tiles
DT = D // P  # 8 d tiles
ET = DK // P  # 2 e tiles

SW = 10240  # proj stream width (5 descriptors x 2048 cols)

_PROGRAM = None


def _stream_offsets(d):
    """(base column of wk chunk, qt c0, qt c1, qt c2/c3 base) for d-block d
    inside its queue's stream tile (blk = d//2 position within the stream)."""
    blk = d // 2
    wk = (blk // 2) * 2048 + (blk % 2) * 1024
    return wk, wk + 512, 4096 + blk * 512, 6144 + blk * 1024


def _build_program():
    nc = bacc.Bacc(None, target_bir_lowering=False, debug=False)

    sa_d = nc.dram_tensor("sa", [P, SW], BF, kind="ExternalInput")
    sb_d = nc.dram_tensor("sb", [P, SW], BF, kind="ExternalInput")
    qn_d = nc.dram_tensor("qn", [4 * P, 4096], BF, kind="ExternalInput")
    wvt_d = nc.dram_tensor("wvt", [2 * P, 4096], BF, kind="ExternalInput")
    y_d = nc.dram_tensor("y", [SQ, D], BF, kind="ExternalOutput")

    with tile.TileContext(nc) as tc:
        with (
            tc.tile_pool(name="pp", bufs=1) as pp,
            tc.tile_pool(name="ps", bufs=1, space="PSUM") as ps,
        ):
            # ---- warmup source: full 128 partitions — K=1 warmups do NOT
            # exercise the PE enough to trigger the DVFS ramp (measured:
            # real matmuls then run at half clock for ~10 instructions) ----
            warm_r = pp.tile([P, 512], BF, tag="warm_r")
            nc.vector.memset(warm_r[:], 1.0)

            # ---- input tiles ----
            sa = pp.tile([P, SW], BF, tag="sa", name="sa")
            sb = pp.tile([P, SW], BF, tag="sb", name="sb")
            qn4 = [
                pp.tile([P, 4096], BF, tag="qn", bufs=4, name=f"qn{t}")
                for t in range(4)
            ]
            wv4 = [
                pp.tile([P, 4096], BF, tag="wvt", bufs=2, name=f"wv{j}")
                for j in range(2)
            ]

            # Input DMAs: 9 descriptors per queue, strict need order. The
            # first 512KB is split in two so the d0 weights+columns land
            # ~1.3us sooner (the early window is HBM-saturated ~130GB/s/q).
            bounds = [0, 1024, 2048, 4096, 6144, 8192, 10240]
            for lo, hi in zip(bounds[:-1], bounds[1:]):
                nc.sync.dma_start(sa[:, lo:hi], sa_d[:, lo:hi])
            for lo, hi in zip(bounds[:-1], bounds[1:]):
                nc.scalar.dma_start(sb[:, lo:hi], sb_d[:, lo:hi])
            for t in (0, 2):
                nc.sync.dma_start(qn4[t][:], qn_d[t * P : (t + 1) * P, :])
            for t in (1, 3):
                nc.scalar.dma_start(qn4[t][:], qn_d[t * P : (t + 1) * P, :])
            nc.sync.dma_start(wv4[0][:], wvt_d[0:P, :])
            nc.scalar.dma_start(wv4[1][:], wvt_d[P : 2 * P, :])

            # ---- warmup matmuls: bridge boot->first-data + DVFS ramp ----
            # 512-col matmuls (~427ns while throttled); sized to end at
            # first-data (~10.3us). A gap here resets the DVFS ramp (worse
            # than a slight overshoot), so err on the long side.
            pwarm = ps.tile([P, 512], F32, tag="sc", bufs=4, name="pwarm")
            for _ in range(WARMUP):
                nc.tensor.matmul(
                    pwarm[:], warm_r[:, :P], warm_r[:], start=True, stop=True
                )

            # ---- constants (after warmup deps; all ready long before use) ----
            ones_f = pp.tile([P, 1], F32, tag="ones_f")
            nc.vector.memset(ones_f[:], 1.0)
            ones = pp.tile([P, 1], FR, tag="ones")
            nc.vector.tensor_copy(ones[:], ones_f[:])
            ident = pp.tile([P, P], F32, tag="ident")
            make_identity(nc, ident[:])
            # Warm the ACT exp table-set (~2.7us first-call cost). Emitted
            # after the scalar-queue DMA issues so the queue arms first.
            warm_act = pp.tile([P, 1], F32, tag="warm_act")
            nc.scalar.activation(
                warm_act[:], ones_f[:], mybir.ActivationFunctionType.Exp
            )

            # slicing helpers into the packed stream tiles
            def strm(d):
                return sa if d % 2 == 0 else sb

            def wk_sl(d, e):
                o = _stream_offsets(d)[0] + e * P
                return strm(d)[:, o : o + P]

            def wq_sl(d, e):
                o = _stream_offsets(d)[0] + 256 + e * P
                return strm(d)[:, o : o + P]

            def qt_sl(d, c):
                offs = _stream_offsets(d)
                if c == 0:
                    base = offs[1]
                elif c == 1:
                    base = offs[2]
                else:
                    base = offs[3] + (c - 2) * 512
                return strm(d)[:, base : base + 512]

            def qn_sl(k, d):
                base = (k % 4) * 1024 + d * P
                return qn4[k // 4][:, base : base + P]

            def wv_sl(d, n):
                base = (d % 4) * 1024 + n * 512
                return wv4[d // 4][:, base : base + 512]

            # ---- persistent on-chip intermediates ----
            kpt = [
                pp.tile([P, S], BF, tag="kpt", bufs=ET, name=f"kpt{e}")
                for e in range(ET)
            ]
            qpt = {
                (e, c): pp.tile([P, CH], BF, tag="qpt", bufs=ET * 2, name=f"qpt{e}_{c}")
                for e in range(ET)
                for c in range(2)
            }
            expt = {}  # (chunk, k) -> bf16 tile, allocated on the fly

            # ---- helpers ----
            def proj_chunk(n, with_qp):
                """kp (and qp) for query/key chunk n, d-major: per qt tile d
                all chunk matmuls run together."""
                pks = [
                    ps.tile([P, 512], F32, tag="sc", bufs=4, name=f"pk{e}_{n}")
                    for e in range(ET)
                ]
                pqs = (
                    [
                        ps.tile([P, 512], F32, tag="acc", bufs=4, name=f"pq{e}_{n}")
                        for e in range(ET)
                    ]
                    if with_qp
                    else []
                )
                for d in range(DT):
                    rhs = qt_sl(d, n)
                    for e in range(ET):
                        nc.tensor.matmul(
                            pks[e][:], wk_sl(d, e), rhs,
                            start=(d == 0), stop=(d == DT - 1),
                        )
                    for e in range(ET):
                        if with_qp:
                            nc.tensor.matmul(
                                pqs[e][:], wq_sl(d, e), rhs,
                                start=(d == 0), stop=(d == DT - 1),
                            )
                for e in range(ET):
                    nc.vector.tensor_copy(kpt[e][:, n * 512 : (n + 1) * 512], pks[e][:])
                for e in range(ET):
                    if with_qp:
                        nc.vector.tensor_copy(qpt[e, n][:], pqs[e][:])

            def scores_block(c, ks):
                """scoresT + exp for key tiles ks of chunk c."""
                for k in ks:
                    sc = ps.tile([P, CH], F32, tag="sc", bufs=4, name=f"sc{c}_{k}")
                    for e in range(ET):
                        nc.tensor.matmul(
                            sc[:],
                            kpt[e][:, k * P : (k + 1) * P],
                            qpt[e, c][:],
                            start=(e == 0),
                            stop=(e == ET - 1),
                        )
                    ex = pp.tile([P, CH], BF, tag="expt", bufs=20, name=f"ex{c}_{k}")
                    nc.scalar.activation(
                        ex[:], sc[:], mybir.ActivationFunctionType.Exp, scale=1.0 / 16.0
                    )
                    expt[c, k] = ex

            def denom_dve(c):
                """DVE part of the denominator: leaf adds pipelined with the
                exp stream, serial fp32 chain tracking them."""
                lvl = [
                    pp.tile([P, CH], F32, tag="dtree", bufs=8, name=f"dt{c}_{i}")
                    for i in range(8)
                ]
                for i in range(8):
                    nc.vector.tensor_tensor(
                        lvl[i][:],
                        expt[c, 2 * i][:],
                        expt[c, 2 * i + 1][:],
                        op=mybir.AluOpType.add,
                    )
                    if i > 0:
                        nc.vector.tensor_tensor(
                            lvl[0][:], lvl[0][:], lvl[i][:], op=mybir.AluOpType.add
                        )
                daccr = pp.tile([P, CH], FR, tag="daccr", bufs=2, name=f"daccr{c}")
                nc.vector.tensor_copy(daccr[:], lvl[0][:])
                return daccr

            def denom_pe(c, daccr):
                """PE part (partition-sum + transpose): emitted later in the
                PE stream so it never head-of-line blocks on the DVE tree."""
                pd = ps.tile([1, CH], F32, tag="sc", bufs=4, name=f"pd{c}")
                nc.tensor.matmul(pd[:], ones[:], daccr[:], start=True, stop=True)
                drow = pp.tile([1, CH], F32, tag="drow", bufs=2, name=f"drow{c}")
                nc.vector.tensor_copy(drow[:], pd[:])
                pt = ps.tile([P, CH // P], F32, tag="sc", bufs=4, name=f"pt{c}")
                for j in range(CH // P):
                    nc.tensor.transpose(
                        pt[:, j : j + 1], drow[:1, j * P : (j + 1) * P], ident[:1, :1]
                    )
                recip = pp.tile([P, CH // P], F32, tag="recip", bufs=2, name=f"recip{c}")
                nc.vector.reciprocal(recip[:], pt[:])
                return recip

            def unnorm_group(c, g, unsb):
                accs = [
                    ps.tile([P, CH], F32, tag="acc", bufs=4, name=f"un{c}_{g}_{i}")
                    for i in range(4)
                ]
                for k in range(KT):
                    for i in range(4):
                        d = g * 4 + i
                        nc.tensor.matmul(
                            accs[i][:],
                            qn_sl(k, d),
                            expt[c, k][:],
                            start=(k == 0),
                            stop=(k == KT - 1),
                        )
                for i in range(4):
                    us = pp.tile([P, CH], BF, tag="unsb", bufs=8, name=f"us{c}_{g}_{i}")
                    # Odd copies ride ACT (has PSUM access, slack between
                    # exp doses) so the y phase never waits on DVE.
                    if i % 2:
                        nc.scalar.activation(
                            us[:], accs[i][:], mybir.ActivationFunctionType.Copy
                        )
                    else:
                        nc.vector.tensor_copy(us[:], accs[i][:])
                    unsb.append(us)

            def y_ms(c, unsb, recip, ms, split_last=False):
                cs = c * CH
                for m in ms:
                    for n in range(D // 512):
                        yb = ps.tile([P, 512], F32, tag="acc", bufs=4, name=f"yb{c}_{m}_{n}")
                        for d in range(DT):
                            nc.tensor.matmul(
                                yb[:],
                                unsb[d][:, m * P : (m + 1) * P],
                                wv_sl(d, n),
                                start=(d == 0),
                                stop=(d == DT - 1),
                            )
                        last = split_last and m == ms[-1] and n == D // 512 - 1
                        # Final block: 256+256 split with one DMA issue per
                        # engine queue so the two 600ns issues overlap.
                        pieces = ((0, 256, nc.sync), (256, 256, nc.scalar)) if last \
                            else ((0, 512, nc.sync),)
                        for h, (off, hw, eng) in enumerate(pieces):
                            ys = pp.tile(
                                [P, hw], BF, tag="ysb", bufs=8, name=f"ys{c}_{m}_{n}_{h}"
                            )
                            # Final block's second mul on ACT (Copy with
                            # per-partition scale): the two halves scale in
                            # parallel instead of serializing on DVE at the
                            # kernel tail.
                            if h:
                                nc.scalar.activation(
                                    ys[:],
                                    yb[:, off : off + hw],
                                    mybir.ActivationFunctionType.Copy,
                                    scale=recip[:, m : m + 1],
                                )
                            else:
                                nc.vector.tensor_scalar_mul(
                                    ys[:], yb[:, off : off + hw], recip[:, m : m + 1]
                                )
                            eng.dma_start(
                                y_d[
                                    cs + m * P : cs + (m + 1) * P,
                                    n * 512 + off : n * 512 + off + hw,
                                ],
                                ys[:],
                            )

            # ---- schedule (trace order == PE priority order) ----
            proj_chunk(0, with_qp=True)
            proj_chunk(1, with_qp=True)
            scores_block(0, range(0, 4))
            proj_chunk(2, with_qp=False)
            scores_block(0, range(4, 8))
            proj_chunk(3, with_qp=False)
            scores_block(0, range(8, 12))
            scores_block(0, range(12, 16))
            daccr0 = denom_dve(0)
            unsb0 = []
            unnorm_group(0, 0, unsb0)
            scores_block(1, range(0, 4))
            recip0 = denom_pe(0, daccr0)
            unnorm_group(0, 1, unsb0)
            scores_block(1, range(4, 8))
            y_ms(0, unsb0, recip0, (0, 1))
            scores_block(1, range(8, 12))
            y_ms(0, unsb0, recip0, (2, 3))
            scores_block(1, range(12, 16))
            daccr1 = denom_dve(1)
            unsb1 = []
            unnorm_group(1, 0, unsb1)
            recip1 = denom_pe(1, daccr1)
            unnorm_group(1, 1, unsb1)
            y_ms(1, unsb1, recip1, (0, 1, 2, 3), split_last=True)

    nc.compile()
    return nc


def build_in_maps(q, Wq, Wk, Wv):
    q = np.asarray(q, dtype=np.float32)

    wqt = np.asarray(Wq, dtype=np.float32).T.astype(NPBF)  # [D, DK]
    wkt = np.asarray(Wk, dtype=np.float32).T.astype(NPBF)
    wkq_full = np.concatenate([wkt, wqt], axis=1)  # [D, 512] rows d*128+p
    # wvt tile J: [128, 4096] cols (d%4)*1024 + e, d=4J..4J+3
    wvT = np.asarray(Wv, dtype=np.float32).T.astype(NPBF)  # [D, D]
    wvt = np.ascontiguousarray(
        wvT.reshape(2, 4, P, 1024).transpose(0, 2, 1, 3).reshape(2 * P, 4096)
    )

    def pack_stream(qtT, ds):
        """[128, SW] stream for d-blocks ds: A1[wk qtc0 x2] A2[same] A3[qt c1
        x4] A4[qt c2c3 x2] A5[same] — linear order == consumption order."""
        blocks = []
        for pair in (ds[0:2], ds[2:4]):  # A1, A2
            for d in pair:
                blocks.append(wkq_full[d * P : (d + 1) * P, :])  # [128,512]
                blocks.append(qtT[d * P : (d + 1) * P, 0:512])
        for d in ds:  # A3
            blocks.append(qtT[d * P : (d + 1) * P, 512:1024])
        for d in ds:  # A4, A5
            blocks.append(qtT[d * P : (d + 1) * P, 1024:2048])
        return np.ascontiguousarray(np.concatenate(blocks, axis=1))

    in_maps = []
    for core in range(N_CORES):
        b, h = divmod(core, 2)
        qb = q[b]
        rolled = np.concatenate(
            [qb[h * SQ : (h + 1) * SQ], qb[(1 - h) * SQ : (2 - h) * SQ]]
        ).astype(NPBF)
        qtT = np.ascontiguousarray(rolled.T)  # [D, S]
        # qn tile t: [128, 4096] cols (k%4)*1024 + d, k=4t..4t+3
        qn_packed = np.ascontiguousarray(
            rolled.reshape(4, 4, P, D).transpose(0, 2, 1, 3).reshape(4 * P, 4096)
        )
        in_maps.append(
            {
                "sa": pack_stream(qtT, (0, 2, 4, 6)),
                "sb": pack_stream(qtT, (1, 3, 5, 7)),
                "qn": qn_packed,
                "wvt": wvt,
            }
        )
    return in_maps


def kernel(q, Wq, Wk, Wv):
    global _PROGRAM
    if _PROGRAM is None:
        _PROGRAM = _build_program()
    nc = _PROGRAM
    in_maps = build_in_maps(q, Wq, Wk, Wv)
    res = run_bass_kernel_spmd(nc, in_maps, list(range(N_CORES)))

    out = np.empty((B, S, D), dtype=np.float32)
    for core in range(N_CORES):
        b, h = divmod(core, 2)
        out[b, h * SQ : (h + 1) * SQ, :] = np.asarray(
            res.results[core]["y"], dtype=np.float32
        )
    return out
